# revision 6
# baseline (speedup 1.0000x reference)
"""MoE layer (B=4,T=2048,D=512,F=1024,E=8,top_k=2) on 8 TRN2 NeuronCores.

Strategy: data-parallel over tokens (1024 tokens/core), weights replicated
(bf16 on host), router in f32 on-device. Sparse capacity-based dispatch:
top-2 routing builds per-expert slot lists via a triangular-matmul prefix
sum + indirect DMA scatters; each expert computes only its C=384 gathered
token slots; outputs are gathered back per token and combined with the
renormalized top-2 weights.
"""
import sys
import types
from contextlib import ExitStack

sys.path.insert(0, "/opt/trn_rl_repo")

import numpy as np
import ml_dtypes

# NTFF profile hook shim: the staged antenv package lacks axon_hooks, which
# bass_utils imports when trace=True under axon. Recreate it from trn_boot.
if "antenv.axon_hooks" not in sys.modules:
    try:
        from trn_agent_boot.trn_boot import _ntff_profile_via_ctypes

        _hook = _ntff_profile_via_ctypes("/opt/axon/libaxon_pjrt.so")
        _mod = types.ModuleType("antenv.axon_hooks")
        _mod.get_axon_ntff_profile_hook = lambda: _hook
        sys.modules["antenv.axon_hooks"] = _mod
    except Exception:
        pass

import concourse.bass as bass
import concourse.tile as tile
from concourse import bacc, mybir
from concourse import bass_utils

bass_utils.upload_artifacts = lambda tmpdir: "local://" + tmpdir

N_CORES = 8
B, T, D, F, E = 4, 2048, 512, 1024, 8
N = B * T              # 8192 tokens total
NT = N // N_CORES      # 1024 tokens per core
P = 128
NTILES = NT // P       # 8 token tiles per core
DT = D // P            # 4 d-tiles
FT = F // P            # 8 f-tiles
F2 = 2 * F
CAP = 384              # slots per expert per core (observed max load: 299)
CT = CAP // P          # slot chunks per expert
EC = E * CAP
EC_PAD = EC + P        # + trash region for (never-expected) overflow

f32 = mybir.dt.float32
bf16 = mybir.dt.bfloat16
u32 = mybir.dt.uint32
i32 = mybir.dt.int32
Alu = mybir.AluOpType
Act = mybir.ActivationFunctionType
Axis = mybir.AxisListType


def _build_moe(tc, out_d, x_d, rwT_d, rb_d, wgu_d, wd_d, phases=3):
    nc = tc.nc
    ctx = ExitStack()
    with ctx:
        # ---------- constants / persistent tiles ----------
        const = ctx.enter_context(tc.tile_pool(name="const", bufs=1))
        identity = const.tile([P, P], f32, name="identity")
        nc.gpsimd.memset(identity[:], 0.0)
        nc.gpsimd.affine_select(
            out=identity[:], in_=identity[:], compare_op=Alu.not_equal, fill=1.0,
            base=0, pattern=[[-1, P]], channel_multiplier=1,
        )
        row_i = const.tile([P, P], i32, name="row_i")
        nc.gpsimd.iota(row_i[:], pattern=[[0, P]], base=0, channel_multiplier=1)
        col_i = const.tile([P, P], i32, name="col_i")
        nc.gpsimd.iota(col_i[:], pattern=[[1, P]], base=0, channel_multiplier=0)
        ltri = const.tile([P, P], f32, name="ltri")
        nc.vector.tensor_tensor(ltri[:], row_i[:], col_i[:], op=Alu.is_lt)
        ones_m = const.tile([P, P], f32, name="ones_m")
        nc.gpsimd.memset(ones_m[:], 1.0)

        rwT_sb = const.tile([P, DT, E], f32, name="rwT_sb")
        nc.sync.dma_start(rwT_sb[:], rwT_d.rearrange("(j p) e -> p j e", p=P))
        rb_row = const.tile([1, E], f32, name="rb_row")
        nc.sync.dma_start(rb_row[:], rb_d[:])
        rb_bcast = const.tile([P, E], f32, name="rb_bcast")
        nc.gpsimd.partition_broadcast(rb_bcast[:], rb_row[:])

        iota_e = const.tile([P, E], i32, name="iota_e")
        nc.gpsimd.iota(iota_e[:], pattern=[[1, E]], base=0, channel_multiplier=0)
        iota_ef = const.tile([P, E], f32, name="iota_ef")
        nc.vector.tensor_copy(iota_ef[:], iota_e[:])

        # per-token routing state kept for the gather-back phase
        m_store = const.tile([P, NTILES, E], f32, name="m_store")
        w1all = const.tile([P, NTILES], f32, name="w1all")
        w2all = const.tile([P, NTILES], f32, name="w2all")
        p1all = const.tile([P, NTILES], i32, name="p1all")
        p2all = const.tile([P, NTILES], i32, name="p2all")

        # DRAM scratch
        dram = ctx.enter_context(tc.tile_pool(name="dram", bufs=1, space="DRAM"))
        g_dram = dram.tile([EC_PAD, 1], i32, name="g_dram")
        y_slots = dram.tile([EC_PAD, D], bf16, name="y_slots")

        # zero-init g_dram so pad slots gather token 0 (harmless)
        gz = const.tile([P, EC_PAD // P], i32, name="gz")
        nc.vector.memset(gz[:], 0)
        nc.sync.dma_start(g_dram.rearrange("(p k) o -> p (k o)", p=P), gz[:])

        # ---------- pools ----------
        xin = ctx.enter_context(tc.tile_pool(name="xin", bufs=3))
        xtf = ctx.enter_context(tc.tile_pool(name="xtf", bufs=2))
        rtr = ctx.enter_context(tc.tile_pool(name="rtr", bufs=2))
        wpool = ctx.enter_context(tc.tile_pool(name="wpool", bufs=2))
        hpool = ctx.enter_context(tc.tile_pool(name="hpool", bufs=2))
        spool = ctx.enter_context(tc.tile_pool(name="spool", bufs=3))
        xgp = ctx.enter_context(tc.tile_pool(name="xgp", bufs=3))
        ygp = ctx.enter_context(tc.tile_pool(name="ygp", bufs=3))
        rpsum = ctx.enter_context(tc.tile_pool(name="rpsum", bufs=2, space="PSUM"))
        gpsum = ctx.enter_context(tc.tile_pool(name="gpsum", bufs=4, space="PSUM"))
        ypsum = ctx.enter_context(tc.tile_pool(name="ypsum", bufs=2, space="PSUM"))

        # ---------- phase 1: router + dispatch ----------
        for i in range(NTILES):
            x_sb = xin.tile([P, D], f32)
            nc.sync.dma_start(x_sb[:], x_d[i * P:(i + 1) * P, :])

            # transpose x tile (f32) for the router matmul
            xTf = xtf.tile([P, DT, P], f32, tag="xTf")
            for j in range(DT):
                pt = rpsum.tile([P, P], f32, tag="rps")
                nc.tensor.transpose(pt[:], x_sb[:, j * P:(j + 1) * P], identity[:])
                nc.scalar.activation(xTf[:, j, :], pt[:], Act.Copy)

            # logits = x @ rwT + rb
            plg = rpsum.tile([P, E], f32, tag="rps")
            for j in range(DT):
                nc.tensor.matmul(
                    plg[:], lhsT=xTf[:, j, :], rhs=rwT_sb[:, j, :],
                    start=(j == 0), stop=(j == DT - 1),
                )
            lg = rtr.tile([P, E], f32, tag="lg")
            nc.vector.tensor_tensor(lg[:], plg[:], rb_bcast[:], op=Alu.add)

            # top-2 (values descending + indices)
            vals8 = rtr.tile([P, 8], f32, tag="vals8")
            idx8 = rtr.tile([P, 8], u32, tag="idx8")
            nc.vector.max(vals8[:], lg[:])
            nc.vector.max_index(idx8[:], vals8[:], lg[:])

            # renormalized weights: w1 = 1/(1+exp(l2-l1)), w2 = 1-w1
            d21 = rtr.tile([P, 1], f32, tag="d21")
            nc.vector.tensor_tensor(d21[:], vals8[:, 1:2], vals8[:, 0:1], op=Alu.subtract)
            z = rtr.tile([P, 1], f32, tag="z")
            nc.scalar.activation(z[:], d21[:], Act.Exp)
            zp1 = rtr.tile([P, 1], f32, tag="zp1")
            nc.vector.tensor_scalar_add(zp1[:], z[:], 1.0)
            w1 = rtr.tile([P, 1], f32, tag="w1")
            nc.vector.reciprocal(w1[:], zp1[:])
            nc.vector.tensor_copy(w1all[:, i:i + 1], w1[:])
            nc.vector.tensor_tensor(w2all[:, i:i + 1], z[:], w1[:], op=Alu.mult)

            # one-hot masks of the two selected experts
            e1f = rtr.tile([P, 1], f32, tag="e1f")
            nc.vector.tensor_copy(e1f[:], idx8[:, 0:1])
            e2f = rtr.tile([P, 1], f32, tag="e2f")
            nc.vector.tensor_copy(e2f[:], idx8[:, 1:2])
            m1 = rtr.tile([P, E], f32, tag="m1")
            nc.vector.tensor_tensor(m1[:], iota_ef[:], e1f[:].to_broadcast([P, E]), op=Alu.is_equal)
            m2 = rtr.tile([P, E], f32, tag="m2")
            nc.vector.tensor_tensor(m2[:], iota_ef[:], e2f[:].to_broadcast([P, E]), op=Alu.is_equal)
            nc.vector.tensor_tensor(m_store[:, i, :], m1[:], m2[:], op=Alu.add)

            # exclusive prefix over all tokens so far:
            #   pos[t,e] = sum_{tiles i'<i} count_{i'}[e]  (all-ones matmul)
            #            + sum_{t'<t in tile i} m[t',e]    (strict-lower matmul)
            ppos = rpsum.tile([P, E], f32, tag="rps")
            for ip in range(i):
                nc.tensor.matmul(ppos[:], lhsT=ones_m[:], rhs=m_store[:, ip, :],
                                 start=(ip == 0), stop=False)
            nc.tensor.matmul(ppos[:], lhsT=ltri[:], rhs=m_store[:, i, :],
                             start=(i == 0), stop=True)
            pos_sb = rtr.tile([P, E], f32, tag="pos_sb")
            nc.vector.tensor_copy(pos_sb[:], ppos[:])

            # slot = e*CAP + pos[t, e_sel]  (clamped to trash on overflow)
            pcons = []
            for (msk, esel, pall) in ((m1, e1f, p1all), (m2, e2f, p2all)):
                eC = rtr.tile([P, 1], f32, tag="eC")
                nc.vector.tensor_scalar_mul(eC[:], esel[:], float(CAP))
                tt = rtr.tile([P, E], f32, tag="tt")
                nc.vector.tensor_tensor(tt[:], pos_sb[:], msk[:], op=Alu.mult)
                psel = rtr.tile([P, 1], f32, tag="psel")
                nc.vector.tensor_reduce(psel[:], tt[:], axis=Axis.X, op=Alu.add)
                slot = rtr.tile([P, 1], f32, tag="slot")
                nc.vector.tensor_tensor(slot[:], eC[:], psel[:], op=Alu.add)
                okm = rtr.tile([P, 1], f32, tag="okm")
                nc.vector.tensor_scalar(okm[:], psel[:], float(CAP), None, op0=Alu.is_lt)
                ovf = rtr.tile([P, 1], f32, tag="ovf")
                nc.vector.tensor_scalar(ovf[:], psel[:], float(CAP), None, op0=Alu.is_ge)
                sl1 = rtr.tile([P, 1], f32, tag="sl1")
                nc.vector.tensor_tensor(sl1[:], slot[:], okm[:], op=Alu.mult)
                sl2 = rtr.tile([P, 1], f32, tag="sl2")
                nc.vector.tensor_scalar_mul(sl2[:], ovf[:], float(EC))
                slc = rtr.tile([P, 1], f32, tag="slc")
                nc.vector.tensor_tensor(slc[:], sl1[:], sl2[:], op=Alu.add)
                nc.vector.tensor_copy(pall[:, i:i + 1], slc[:])
                # contiguous copy for the indirect-DMA offset operand
                pcon = rtr.tile([P, 1], i32, tag="pcon" + ("1" if pall is p1all else "2"))
                nc.vector.tensor_copy(pcon[:], slc[:])
                pcons.append(pcon)

            # scatter token ids into the per-expert slot table
            tok = rtr.tile([P, 1], i32, tag="tok")
            nc.gpsimd.iota(tok[:], pattern=[[1, 1]], base=i * P, channel_multiplier=1)
            for pcon in pcons:
                nc.gpsimd.indirect_dma_start(
                    out=g_dram[:],
                    out_offset=bass.IndirectOffsetOnAxis(ap=pcon[:, 0:1], axis=0),
                    in_=tok[:], in_offset=None,
                )

        if phases < 3:
            for i in range(NTILES):
                nc.sync.dma_start(out_d[i * P:(i + 1) * P, :], x_d[i * P:(i + 1) * P, :])
        # ---------- phase 2: experts (sparse, CAP slots each) ----------
        for e in (range(E) if phases >= 2 else []):
            wgu_sb = wpool.tile([P, DT, F2], bf16, tag="wgu")
            nc.sync.dma_start(wgu_sb[:], wgu_d[e].rearrange("(j p) f -> p j f", p=P))
            wd_sb = wpool.tile([P, FT, D], bf16, tag="wd")
            nc.sync.dma_start(wd_sb[:], wd_d[e].rearrange("(j p) f -> p j f", p=P))

            # gather this expert's tokens and transpose them
            xt_e = xgp.tile([P, DT, CAP], bf16, tag="xt_e")
            for s in range(CT):
                gidx = xgp.tile([P, 1], i32, tag="gidx")
                nc.sync.dma_start(gidx[:], g_dram[e * CAP + s * P: e * CAP + (s + 1) * P, :])
                xg = xgp.tile([P, D], f32, tag="xg")
                nc.gpsimd.indirect_dma_start(
                    out=xg[:], out_offset=None,
                    in_=x_d[:], in_offset=bass.IndirectOffsetOnAxis(ap=gidx[:, 0:1], axis=0),
                )
                for j in range(DT):
                    pt = rpsum.tile([P, P], f32, tag="rps")
                    nc.tensor.transpose(pt[:], xg[:, j * P:(j + 1) * P], identity[:])
                    nc.vector.tensor_copy(xt_e[:, j, s * P:(s + 1) * P], pt[:])

            # gate/up matmuls + SwiGLU -> hT (transposed, bf16)
            hT = hpool.tile([P, FT, CAP], bf16, tag="hT")
            for ft in range(FT):
                pg = gpsum.tile([P, CAP], f32, tag="gu")
                for j in range(DT):
                    nc.tensor.matmul(
                        pg[:], lhsT=wgu_sb[:, j, ft * P:(ft + 1) * P],
                        rhs=xt_e[:, j, :],
                        start=(j == 0), stop=(j == DT - 1),
                    )
                pu = gpsum.tile([P, CAP], f32, tag="gu")
                for j in range(DT):
                    nc.tensor.matmul(
                        pu[:], lhsT=wgu_sb[:, j, (ft + FT) * P:(ft + FT + 1) * P],
                        rhs=xt_e[:, j, :],
                        start=(j == 0), stop=(j == DT - 1),
                    )
                sg = spool.tile([P, CAP], f32, tag="sg")
                nc.scalar.activation(sg[:], pg[:], Act.Silu)
                nc.vector.tensor_tensor(hT[:, ft, :], sg[:], pu[:], op=Alu.mult)

            # down-projection back to token-major, write expert slots
            for s in range(CT):
                py = ypsum.tile([P, D], f32, tag="py")
                for ft in range(FT):
                    nc.tensor.matmul(
                        py[:], lhsT=hT[:, ft, s * P:(s + 1) * P],
                        rhs=wd_sb[:, ft, :],
                        start=(ft == 0), stop=(ft == FT - 1),
                    )
                ybf = ygp.tile([P, D], bf16, tag="ybf")
                nc.scalar.activation(ybf[:], py[:], Act.Copy)
                nc.sync.dma_start(
                    y_slots[e * CAP + s * P: e * CAP + (s + 1) * P, :], ybf[:])

        # ---------- phase 3: gather back + combine ----------
        for i in (range(NTILES) if phases >= 3 else []):
            q1 = ygp.tile([P, 1], i32, tag="q1")
            nc.vector.tensor_copy(q1[:], p1all[:, i:i + 1])
            q2 = ygp.tile([P, 1], i32, tag="q2")
            nc.vector.tensor_copy(q2[:], p2all[:, i:i + 1])
            y1 = ygp.tile([P, D], bf16, tag="y1")
            nc.gpsimd.indirect_dma_start(
                out=y1[:], out_offset=None,
                in_=y_slots[:], in_offset=bass.IndirectOffsetOnAxis(ap=q1[:, 0:1], axis=0),
            )
            y2 = ygp.tile([P, D], bf16, tag="y2")
            nc.gpsimd.indirect_dma_start(
                out=y2[:], out_offset=None,
                in_=y_slots[:], in_offset=bass.IndirectOffsetOnAxis(ap=q2[:, 0:1], axis=0),
            )
            t1 = spool.tile([P, D], f32, tag="t1")
            nc.scalar.activation(t1[:], y1[:], Act.Copy, scale=w1all[:, i:i + 1])
            t2 = spool.tile([P, D], f32, tag="t2")
            nc.vector.tensor_scalar_mul(t2[:], y2[:], w2all[:, i:i + 1])
            ot = spool.tile([P, D], f32, tag="ot")
            nc.vector.tensor_tensor(ot[:], t1[:], t2[:], op=Alu.add)
            nc.sync.dma_start(out_d[i * P:(i + 1) * P, :], ot[:])


_compiled = {}


def _get_compiled(phases=3):
    global _compiled
    if _compiled.get(phases) is None:
        nc = bacc.Bacc("TRN2", target_bir_lowering=False, debug=False,
                       num_devices=N_CORES)
        x_d = nc.dram_tensor("x", [NT, D], f32, kind="ExternalInput").ap()
        rwT_d = nc.dram_tensor("rwT", [D, E], f32, kind="ExternalInput").ap()
        rb_d = nc.dram_tensor("rb", [1, E], f32, kind="ExternalInput").ap()
        wgu_d = nc.dram_tensor("wgu", [E, D, F2], bf16, kind="ExternalInput").ap()
        wd_d = nc.dram_tensor("wd", [E, F, D], bf16, kind="ExternalInput").ap()
        out_d = nc.dram_tensor("out", [NT, D], f32, kind="ExternalOutput").ap()
        with tile.TileContext(nc) as tc:
            _build_moe(tc, out_d, x_d, rwT_d, rb_d, wgu_d, wd_d, phases=phases)
        nc.compile()
        _compiled[phases] = nc
    return _compiled[phases]


def _run(inputs, trace=False, trace_cores=None, phases=3):
    x = np.ascontiguousarray(np.asarray(inputs["x"], dtype=np.float32)).reshape(N, D)
    router_w = np.asarray(inputs["router_w"], dtype=np.float32)
    router_b = np.asarray(inputs["router_b"], dtype=np.float32)
    wgu = np.asarray(inputs["w_gate_up"], dtype=np.float32)
    wd = np.asarray(inputs["w_down"], dtype=np.float32)
    assert int(inputs.get("top_k", 2)) == 2

    rwT = np.ascontiguousarray(router_w.T)                      # [D, E] f32
    rb = np.ascontiguousarray(router_b.reshape(1, E))           # [1, E] f32
    wgu_bf = wgu.astype(ml_dtypes.bfloat16)                     # [E, D, 2F]
    wd_bf = wd.astype(ml_dtypes.bfloat16)                       # [E, F, D]

    nc = _get_compiled(phases)
    in_maps = []
    for c in range(N_CORES):
        in_maps.append({
            "x": x[c * NT:(c + 1) * NT],
            "rwT": rwT,
            "rb": rb,
            "wgu": wgu_bf,
            "wd": wd_bf,
        })
    res = bass_utils.run_bass_kernel_spmd(
        nc, in_maps, core_ids=list(range(N_CORES)),
        trace=trace, trace_cores=trace_cores,
    )
    out = np.concatenate([res.results[c]["out"] for c in range(N_CORES)], axis=0)
    return out.reshape(B, T, D), res


def kernel(**inputs):
    out, _ = _run(inputs)
    return out
